# revision 14
# baseline (speedup 1.0000x reference)
"""Trainium2 Bass kernel for nn_AdaptedGaussianConditional (VQ codebook
quantize/dequantize), SPMD over 8 NeuronCores, data-parallel over batch.

Math: for v = inputs - means, the reference assigns
  symbols(v) = #{i in 0..254 : v >= t_i}
where t_i is the exact fp32 decision boundary between symbol i and i+1
(computed on host by bisecting the reference predicate), and
  dequant = unique_values[symbols] + means.

Device algorithm (pure elementwise fp32 on the Vector engine, no gather):
  * count masks (v > c_i), c_i = pred(t_i), via fused scalar_tensor_tensor
    chains: acc' = (v is_gt c_i) add acc — one DVE instruction/threshold.
  * thresholds are partitioned into W weight classes (gap values quantized
    to the dyadic grid Q); class w's count is accumulated separately, then
    folded as merged += (val_w + delta)*count_w with delta = 2^-17 a
    sub-ulp tag. Host-side error feedback in the class assignment bounds
    the cumulative dequant error by ~half a quantization step. All fold
    arithmetic is exact in fp32 (every term is a multiple of 2^-17 and the
    total stays far below 2^24 ulps).
  * merged splits exactly: round(merged/Q) is the quantized codebook
    offset (frac < 0.5 always), the fractional tag recovers symbols
    exactly; symbols is bit-identical to the reference.
"""

import numpy as np

from concourse import bass, mybir
from concourse.bass_utils import run_bass_kernel_spmd

# Problem shape (hardcoded per spec).
B, CC, HH, WW = 16, 192, 64, 64
L = 256
NCORES = 8
P = 128
F_TILE = 3072
ELEMS_PER_CORE = (B // NCORES) * CC * HH * WW          # 1,572,864
FREE_PER_PART = ELEMS_PER_CORE // P                    # 12,288
NTILES = FREE_PER_PART // F_TILE                       # 4

QLOG2 = -5
Q = float(2.0 ** QLOG2)           # dequant value quantization step
DELTA = float(2.0 ** -17)         # sub-ulp symbol tag
HUGE = float(np.float32(3.0e38))  # "never true" threshold pad

f32 = mybir.dt.float32
i32 = mybir.dt.int32


# --------------------------------------------------------------------------
# Host-side planning: exact boundaries + weight classes
# --------------------------------------------------------------------------
def _f2k(x: np.ndarray) -> np.ndarray:
    """Monotone uint32 key for float32 total order (negatives -> [0, 2^31))."""
    i = x.astype(np.float32).view(np.int32).astype(np.int64)
    return np.where(i >= 0, i + 0x80000000, -1 - i).astype(np.uint64)


def _k2f(k: np.ndarray) -> np.ndarray:
    k = k.astype(np.int64)
    i = np.where(k >= 0x80000000, k - 0x80000000, -1 - k)
    return i.astype(np.int32).view(np.float32)


def _ref_symbols_fp32(v: np.ndarray, uv: np.ndarray) -> np.ndarray:
    """Exact fp32 replica of the reference's nearest-symbol computation."""
    v = v.astype(np.float32)
    idx = np.searchsorted(uv, v, side="left")
    idx = np.clip(idx, 1, L - 1)
    left = uv[idx - 1]
    right = uv[idx]
    dl = np.abs((v - left).astype(np.float32))
    dr = np.abs((v - right).astype(np.float32))
    return np.where(dl <= dr, idx - 1, idx).astype(np.int32)


def _exact_boundaries(uv: np.ndarray) -> np.ndarray:
    """t[i] = smallest fp32 v with ref symbol >= i+1. Vectorized bisection
    on the fp32 total-order keys, all 255 boundaries at once."""
    lo = _f2k(uv[:-1])      # symbol(uv[i]) == i
    hi = _f2k(uv[1:])       # symbol(uv[i+1]) == i+1
    tgt = np.arange(1, L)
    # invariant: symbol(k2f(lo)) < tgt <= symbol(k2f(hi))
    while True:
        gap = hi - lo
        if (gap <= 1).all():
            break
        mid = lo + gap // 2
        sm = _ref_symbols_fp32(_k2f(mid), uv)
        ge = sm >= tgt
        hi = np.where(ge, mid, hi)
        lo = np.where(ge, lo, mid)
    return _k2f(hi)


def _plan(uv: np.ndarray):
    """Returns (thresholds c_i, class_of_i, class values, W)."""
    uv = uv.astype(np.float32)
    t = _exact_boundaries(uv)
    # c_i = pred(t_i): (v > c_i) <=> v >= t_i for all fp32 v
    c = np.nextafter(t, np.float32(-np.inf), dtype=np.float32)

    # validate the count identity  #{i: v >= t_i} == ref_symbols(v)  on
    # probes straddling every decision boundary (exactness insurance)
    probes = np.concatenate([t, c, uv, np.nextafter(uv, np.float32(np.inf),
                                                    dtype=np.float32)])
    cnt = (probes[:, None] > c[None, :]).sum(axis=1).astype(np.int32)
    ref = _ref_symbols_fp32(probes, uv)
    assert np.array_equal(cnt, ref), "threshold plan failed validation"

    gaps = (uv[1:].astype(np.float64) - uv[:-1].astype(np.float64))
    gmax = float(gaps.max())
    W = int(np.ceil(gmax / Q)) + 2
    vals = np.arange(W, dtype=np.float64) * Q
    # error-feedback assignment: bounded cumulative reconstruction error
    cls = np.zeros(L - 1, dtype=np.int64)
    err = 0.0
    for i in range(L - 1):
        w = int(np.clip(np.round((gaps[i] - err) / Q), 0, W - 1))
        cls[i] = w
        err += vals[w] - gaps[i]
    return c, cls, vals, W


def _host_check_plan(uv, c, cls, vals):
    """Max abs dequant reconstruction error over all 256 symbols."""
    recon = np.zeros(L, dtype=np.float64)
    recon[1:] = np.cumsum(vals[cls])
    recon += float(uv[0])
    return np.abs(recon - uv.astype(np.float64)).max()


# --------------------------------------------------------------------------
# Bass graph
# --------------------------------------------------------------------------
def _build(c: np.ndarray, cls: np.ndarray, vals: np.ndarray, W: int,
           uv0: float) -> bass.Bass:
    per_class = [list(np.asarray(c)[cls == w]) for w in range(W)]

    nc = bass.Bass()
    a_ext = nc.dram_tensor("a", [P, FREE_PER_PART], f32, kind="ExternalInput").ap()
    b_ext = nc.dram_tensor("b", [P, FREE_PER_PART], f32, kind="ExternalInput").ap()
    d_ext = nc.dram_tensor("dq", [P, FREE_PER_PART], f32, kind="ExternalOutput").ap()
    s_ext = nc.dram_tensor("sym", [P, FREE_PER_PART], i32, kind="ExternalOutput").ap()

    with (
        nc.Block() as block,
        nc.semaphore("dma_in_sem") as dma_in_sem,
        nc.semaphore("dma_out_sem") as dma_out_sem,
        nc.semaphore("cmp_sem") as cmp_sem,
        nc.sbuf_tensor("a_sb0", [P, F_TILE], f32) as a_sb0,
        nc.sbuf_tensor("a_sb1", [P, F_TILE], f32) as a_sb1,
        nc.sbuf_tensor("b_sb0", [P, F_TILE], f32) as b_sb0,
        nc.sbuf_tensor("b_sb1", [P, F_TILE], f32) as b_sb1,
        nc.sbuf_tensor("v_sb", [P, F_TILE], f32) as v_sb,
        nc.sbuf_tensor("acc_a", [P, F_TILE], f32) as acc_a,
        nc.sbuf_tensor("acc_b", [P, F_TILE], f32) as acc_b,
        nc.sbuf_tensor("mrg_a", [P, F_TILE], f32) as mrg_a,
        nc.sbuf_tensor("mrg_b", [P, F_TILE], f32) as mrg_b,
        nc.sbuf_tensor("tmp_a", [P, F_TILE], f32) as tmp_a,
        nc.sbuf_tensor("tmp_b", [P, F_TILE], f32) as tmp_b,
        nc.sbuf_tensor("d_sb0", [P, F_TILE], f32) as d_sb0,
        nc.sbuf_tensor("d_sb1", [P, F_TILE], f32) as d_sb1,
        nc.sbuf_tensor("si_sb0", [P, F_TILE], i32) as si_sb0,
        nc.sbuf_tensor("si_sb1", [P, F_TILE], i32) as si_sb1,
    ):
        a_sb = [a_sb0, a_sb1]
        b_sb = [b_sb0, b_sb1]
        d_sb = [d_sb0, d_sb1]
        si_sb = [si_sb0, si_sb1]

        @block.sync
        def _(sync):
            in_ctr = 0

            def dma_in(t):
                nonlocal in_ctr
                sl = slice(t * F_TILE, (t + 1) * F_TILE)
                sync.dma_start(a_sb[t % 2].ap(), a_ext[:, sl]).then_inc(dma_in_sem, 16)
                sync.dma_start(b_sb[t % 2].ap(), b_ext[:, sl]).then_inc(dma_in_sem, 16)
                in_ctr += 32

            dma_in(0)
            if NTILES > 1:
                dma_in(1)
            out_ctr = 0
            for t in range(NTILES):
                sync.wait_ge(cmp_sem, t + 1)
                sl = slice(t * F_TILE, (t + 1) * F_TILE)
                sync.dma_start(d_ext[:, sl], d_sb[t % 2].ap()).then_inc(dma_out_sem, 16)
                sync.dma_start(s_ext[:, sl], si_sb[t % 2].ap()).then_inc(dma_out_sem, 16)
                out_ctr += 32
                if t + 2 < NTILES:
                    dma_in(t + 2)
            sync.wait_ge(dma_out_sem, out_ctr)

        @block.vector
        def _(vector):
            uv0_f = uv0
            for t in range(NTILES):
                vector.wait_ge(dma_in_sem, 32 * (t + 1))
                if t >= 2:
                    vector.wait_ge(dma_out_sem, 32 * (t - 1))
                va = v_sb.ap()
                # v = a - b
                vector.tensor_tensor(va, a_sb[t % 2].ap(), b_sb[t % 2].ap(),
                                     mybir.AluOpType.subtract)
                # class chains + incremental merged fold
                mrg = [mrg_a, mrg_b]
                mi = 0
                first_class = True
                for w in range(W):
                    th = per_class[w]
                    if len(th) == 0:
                        continue
                    accs = [acc_a, acc_b]
                    ai = 0
                    # seed: acc = (v > th0)   (tensor_scalar, 2x fp32)
                    vector.tensor_scalar(accs[ai].ap(), va, float(th[0]), None,
                                         mybir.AluOpType.is_gt)
                    # chain: acc' = (v > th_j) + acc
                    for t_j in th[1:]:
                        vector.scalar_tensor_tensor(
                            accs[1 - ai].ap(), va, float(t_j), accs[ai].ap(),
                            mybir.AluOpType.is_gt, mybir.AluOpType.add)
                        ai = 1 - ai
                    wv = float(np.float32(vals[w] + DELTA))
                    if first_class:
                        # merged = acc * (val_w + delta)
                        vector.tensor_scalar_mul(mrg[mi].ap(), accs[ai].ap(), wv)
                        first_class = False
                    else:
                        # merged' = acc * (val_w + delta) + merged
                        vector.scalar_tensor_tensor(
                            mrg[1 - mi].ap(), accs[ai].ap(), wv, mrg[mi].ap(),
                            mybir.AluOpType.mult, mybir.AluOpType.add)
                        mi = 1 - mi
                # extraction
                # t64 = merged * 64
                vector.tensor_scalar_mul(tmp_a.ap(), mrg[mi].ap(), 1.0 / Q)
                # ti (i32, round-to-nearest == floor since frac < 0.5)
                vector.tensor_copy(si_sb[t % 2].ap(), tmp_a.ap())
                # back to f32
                vector.tensor_copy(tmp_b.ap(), si_sb[t % 2].ap())
                # frac = t64 - ti
                vector.tensor_tensor(v_sb.ap(), tmp_a.ap(), tmp_b.ap(),
                                     mybir.AluOpType.subtract)
                # symbols = frac * 2048 (exact integer) -> i32
                vector.tensor_scalar_mul(si_sb[t % 2].ap(), v_sb.ap(),
                                         Q / DELTA)
                # d = ti * Q + uv0
                vector.tensor_scalar(tmp_a.ap(), tmp_b.ap(), Q, uv0_f,
                                     mybir.AluOpType.mult, mybir.AluOpType.add)
                # d += means
                vector.tensor_tensor(d_sb[t % 2].ap(), tmp_a.ap(), b_sb[t % 2].ap(),
                                     mybir.AluOpType.add).then_inc(cmp_sem, 1)

    return nc


# --------------------------------------------------------------------------
# Public entry point
# --------------------------------------------------------------------------
_CACHE: dict[bytes, bass.Bass] = {}


def _get_nc(uv: np.ndarray) -> bass.Bass:
    key = uv.tobytes()
    if key not in _CACHE:
        c, cls, vals, W = _plan(uv)
        _CACHE[key] = _build(c, cls, vals, W, float(np.float32(uv[0])))
    return _CACHE[key]


def kernel(inputs: np.ndarray, means: np.ndarray, unique_values: np.ndarray):
    inputs = np.ascontiguousarray(np.asarray(inputs, dtype=np.float32))
    means = np.ascontiguousarray(np.asarray(means, dtype=np.float32))
    uv = np.ascontiguousarray(np.asarray(unique_values, dtype=np.float32))

    nc = _get_nc(uv)

    bpc = B // NCORES
    in_maps = []
    for cid in range(NCORES):
        a = inputs[cid * bpc:(cid + 1) * bpc].reshape(P, FREE_PER_PART)
        b = means[cid * bpc:(cid + 1) * bpc].reshape(P, FREE_PER_PART)
        in_maps.append({"a": np.ascontiguousarray(a),
                        "b": np.ascontiguousarray(b)})

    res = run_bass_kernel_spmd(nc, in_maps, core_ids=list(range(NCORES)))

    dq = np.empty((B, CC, HH, WW), dtype=np.float32)
    sym = np.empty((B, CC, HH, WW), dtype=np.int32)
    for cid in range(NCORES):
        r = res.results[cid]
        dq[cid * bpc:(cid + 1) * bpc] = r["dq"].reshape(bpc, CC, HH, WW)
        sym[cid * bpc:(cid + 1) * bpc] = r["sym"].reshape(bpc, CC, HH, WW)
    return dq, sym


# revision 16
# speedup vs baseline: 1.2083x; 1.2083x over previous
"""Trainium2 Bass kernel for nn_AdaptedGaussianConditional (VQ codebook
quantize/dequantize), SPMD over 8 NeuronCores, data-parallel over batch.

Math: for v = inputs - means, the reference assigns
  symbols(v) = #{i in 0..254 : v >= t_i}
where t_i is the exact fp32 decision boundary between symbol i and i+1
(computed on host by bisecting the reference predicate), and
  dequant = unique_values[symbols] + means.

Device algorithm (pure elementwise fp32 on the Vector engine, no gather):
  * count masks (v > c_i), c_i = pred(t_i), via fused scalar_tensor_tensor
    chains: acc' = (v is_gt c_i) add acc — one DVE instruction/threshold.
  * thresholds are partitioned into W weight classes (gap values quantized
    to the dyadic grid Q); class w's count is accumulated separately, then
    folded as merged += (val_w + delta)*count_w with delta = 2^-17 a
    sub-ulp tag. Host-side error feedback in the class assignment bounds
    the cumulative dequant error by ~half a quantization step. All fold
    arithmetic is exact in fp32 (every term is a multiple of 2^-17 and the
    total stays far below 2^24 ulps).
  * merged splits exactly: round(merged/Q) is the quantized codebook
    offset (frac < 0.5 always), the fractional tag recovers symbols
    exactly; symbols is bit-identical to the reference.
"""

import numpy as np

from concourse import bass, mybir
from concourse.bass_utils import run_bass_kernel_spmd

# Problem shape (hardcoded per spec).
B, CC, HH, WW = 16, 192, 64, 64
L = 256
NCORES = 8
P = 128
F_TILE = 2048
ELEMS_PER_CORE = (B // NCORES) * CC * HH * WW          # 1,572,864
FREE_PER_PART = ELEMS_PER_CORE // P                    # 12,288
NTILES = FREE_PER_PART // F_TILE                       # 4

QLOG2 = -5
Q = float(2.0 ** QLOG2)           # dequant value quantization step
DELTA = float(2.0 ** -17)         # sub-ulp symbol tag
HUGE = float(np.float32(3.0e38))  # "never true" threshold pad
N_GPS = 68                        # thresholds offloaded to GPSIMD

f32 = mybir.dt.float32
i32 = mybir.dt.int32


# --------------------------------------------------------------------------
# Host-side planning: exact boundaries + weight classes
# --------------------------------------------------------------------------
def _f2k(x: np.ndarray) -> np.ndarray:
    """Monotone uint32 key for float32 total order (negatives -> [0, 2^31))."""
    i = x.astype(np.float32).view(np.int32).astype(np.int64)
    return np.where(i >= 0, i + 0x80000000, -1 - i).astype(np.uint64)


def _k2f(k: np.ndarray) -> np.ndarray:
    k = k.astype(np.int64)
    i = np.where(k >= 0x80000000, k - 0x80000000, -1 - k)
    return i.astype(np.int32).view(np.float32)


def _ref_symbols_fp32(v: np.ndarray, uv: np.ndarray) -> np.ndarray:
    """Exact fp32 replica of the reference's nearest-symbol computation."""
    v = v.astype(np.float32)
    idx = np.searchsorted(uv, v, side="left")
    idx = np.clip(idx, 1, L - 1)
    left = uv[idx - 1]
    right = uv[idx]
    dl = np.abs((v - left).astype(np.float32))
    dr = np.abs((v - right).astype(np.float32))
    return np.where(dl <= dr, idx - 1, idx).astype(np.int32)


def _exact_boundaries(uv: np.ndarray) -> np.ndarray:
    """t[i] = smallest fp32 v with ref symbol >= i+1. Vectorized bisection
    on the fp32 total-order keys, all 255 boundaries at once."""
    lo = _f2k(uv[:-1])      # symbol(uv[i]) == i
    hi = _f2k(uv[1:])       # symbol(uv[i+1]) == i+1
    tgt = np.arange(1, L)
    # invariant: symbol(k2f(lo)) < tgt <= symbol(k2f(hi))
    while True:
        gap = hi - lo
        if (gap <= 1).all():
            break
        mid = lo + gap // 2
        sm = _ref_symbols_fp32(_k2f(mid), uv)
        ge = sm >= tgt
        hi = np.where(ge, mid, hi)
        lo = np.where(ge, lo, mid)
    return _k2f(hi)


def _plan(uv: np.ndarray):
    """Returns (thresholds c_i, class_of_i, class values, W)."""
    uv = uv.astype(np.float32)
    t = _exact_boundaries(uv)
    # c_i = pred(t_i): (v > c_i) <=> v >= t_i for all fp32 v
    c = np.nextafter(t, np.float32(-np.inf), dtype=np.float32)

    # validate the count identity  #{i: v >= t_i} == ref_symbols(v)  on
    # probes straddling every decision boundary (exactness insurance)
    probes = np.concatenate([t, c, uv, np.nextafter(uv, np.float32(np.inf),
                                                    dtype=np.float32)])
    cnt = (probes[:, None] > c[None, :]).sum(axis=1).astype(np.int32)
    ref = _ref_symbols_fp32(probes, uv)
    assert np.array_equal(cnt, ref), "threshold plan failed validation"

    gaps = (uv[1:].astype(np.float64) - uv[:-1].astype(np.float64))
    gmax = float(gaps.max())
    W = int(np.ceil(gmax / Q)) + 2
    vals = np.arange(W, dtype=np.float64) * Q
    # error-feedback assignment: bounded cumulative reconstruction error
    cls = np.zeros(L - 1, dtype=np.int64)
    err = 0.0
    for i in range(L - 1):
        w = int(np.clip(np.round((gaps[i] - err) / Q), 0, W - 1))
        cls[i] = w
        err += vals[w] - gaps[i]
    return c, cls, vals, W


def _host_check_plan(uv, c, cls, vals):
    """Max abs dequant reconstruction error over all 256 symbols."""
    recon = np.zeros(L, dtype=np.float64)
    recon[1:] = np.cumsum(vals[cls])
    recon += float(uv[0])
    return np.abs(recon - uv.astype(np.float64)).max()


# --------------------------------------------------------------------------
# Bass graph
# --------------------------------------------------------------------------
def _build(c: np.ndarray, cls: np.ndarray, vals: np.ndarray, W: int,
           uv0: float) -> bass.Bass:
    # Per-threshold merged weights (val_class + DELTA), exact fp32 multiples
    # of 2^-17. GPSIMD-assigned thresholds carry their weight per-op (dual-op
    # tensor_scalar); DVE thresholds stay grouped in per-class count chains.
    wv_of = np.float32(vals[cls] + DELTA)
    order = np.argsort(cls, kind="stable")          # class-major order
    gps_pick = np.zeros(L - 1, dtype=bool)
    if N_GPS > 0:
        stride = max(1, (L - 1) // N_GPS)
        gps_pick[order[::stride][:N_GPS]] = True
    gps_list = [(float(c[i]), float(wv_of[i])) for i in range(L - 1) if gps_pick[i]]
    per_class = [list(np.asarray(c)[(cls == w) & ~gps_pick]) for w in range(W)]

    nc = bass.Bass()
    a_ext = nc.dram_tensor("a", [P, FREE_PER_PART], f32, kind="ExternalInput").ap()
    b_ext = nc.dram_tensor("b", [P, FREE_PER_PART], f32, kind="ExternalInput").ap()
    d_ext = nc.dram_tensor("dq", [P, FREE_PER_PART], f32, kind="ExternalOutput").ap()
    s_ext = nc.dram_tensor("sym", [P, FREE_PER_PART], i32, kind="ExternalOutput").ap()

    from contextlib import ExitStack
    ctx = ExitStack()
    with ctx:
        sem = lambda n: ctx.enter_context(nc.semaphore(n))
        sb = lambda n: ctx.enter_context(nc.sbuf_tensor(n, [P, F_TILE], f32))
        sbi = lambda n: ctx.enter_context(nc.sbuf_tensor(n, [P, F_TILE], i32))
        block = ctx.enter_context(nc.Block())
        dma_in_sem = sem("dma_in_sem")
        dma_out_sem = sem("dma_out_sem")
        cmp_sem = sem("cmp_sem")
        v_sem = sem("v_sem")
        gp_sem = sem("gp_sem")
        a_sb0, a_sb1 = sb("a_sb0"), sb("a_sb1")
        b_sb0, b_sb1 = sb("b_sb0"), sb("b_sb1")
        v_sb0, v_sb1 = sb("v_sb0"), sb("v_sb1")
        mrg_a, mrg_b = sb("mrg_a"), sb("mrg_b")
        gm_sb = sb("gm_sb")
        mg_s0, mg_s1 = sb("mg_s0"), sb("mg_s1")
        mg_f0, mg_f1 = sb("mg_f0"), sb("mg_f1")
        tmp_a, tmp_b = sb("tmp_a"), sb("tmp_b")
        d_sb0, d_sb1 = sb("d_sb0"), sb("d_sb1")
        si_sb0, si_sb1 = sbi("si_sb0"), sbi("si_sb1")
        pacc = ctx.enter_context(nc.psum_tensor("pacc", [P, F_TILE], f32))
        a_sb = [a_sb0, a_sb1]
        b_sb = [b_sb0, b_sb1]
        v_sb = [v_sb0, v_sb1]
        d_sb = [d_sb0, d_sb1]
        si_sb = [si_sb0, si_sb1]
        mg_f = [mg_f0, mg_f1]
        mg_s = [mg_s0, mg_s1]

        @block.sync
        def _(sync):
            def dma_in(t):
                sl = slice(t * F_TILE, (t + 1) * F_TILE)
                sync.dma_start(a_sb[t % 2].ap(), a_ext[:, sl]).then_inc(dma_in_sem, 16)
                sync.dma_start(b_sb[t % 2].ap(), b_ext[:, sl]).then_inc(dma_in_sem, 16)

            dma_in(0)
            if NTILES > 1:
                dma_in(1)
            out_ctr = 0
            for t in range(NTILES):
                sync.wait_ge(cmp_sem, t + 1)
                sl = slice(t * F_TILE, (t + 1) * F_TILE)
                sync.dma_start(d_ext[:, sl], d_sb[t % 2].ap()).then_inc(dma_out_sem, 16)
                sync.dma_start(s_ext[:, sl], si_sb[t % 2].ap()).then_inc(dma_out_sem, 16)
                out_ctr += 32
                if t + 2 < NTILES:
                    dma_in(t + 2)
            sync.wait_ge(dma_out_sem, out_ctr)

        if gps_list:

            @block.gpsimd
            def _(gpsimd):
                for t in range(NTILES):
                    gpsimd.wait_ge(v_sem, t + 1)
                    va = v_sb[t % 2].ap()
                    n = len(gps_list)
                    # seed: mg = (v > c0) * w0
                    c0, w0 = gps_list[0]
                    dst0 = mg_f[t % 2] if n == 1 else mg_s[0]
                    ins = gpsimd.tensor_scalar(dst0.ap(), va, c0, w0,
                                               mybir.AluOpType.is_gt,
                                               mybir.AluOpType.mult)
                    si_ = 0
                    for j in range(1, n):
                        cj, wj = gps_list[j]
                        gpsimd.tensor_scalar(gm_sb.ap(), va, cj, wj,
                                             mybir.AluOpType.is_gt,
                                             mybir.AluOpType.mult)
                        dst = mg_f[t % 2] if j == n - 1 else mg_s[1 - si_]
                        ins = gpsimd.tensor_tensor(dst.ap(), gm_sb.ap(),
                                                   mg_s[si_].ap(),
                                                   mybir.AluOpType.add)
                        si_ = 1 - si_
                    ins.then_inc(gp_sem, 1)

        @block.vector
        def _(vector):
            uv0_f = uv0
            mrg = [mrg_a, mrg_b]
            for t in range(NTILES):
                vector.wait_ge(dma_in_sem, 32 * (t + 1))
                if t >= 2:
                    vector.wait_ge(dma_out_sem, 32 * (t - 1))
                if t == 0:
                    vector.tensor_tensor(v_sb[0].ap(), a_sb[0].ap(), b_sb[0].ap(),
                                         mybir.AluOpType.subtract).then_inc(v_sem, 1)
                va = v_sb[t % 2].ap()
                # DVE class chains: count accumulates in-place in PSUM, so the
                # shared SBUF port stays free for GPSIMD's concurrent chains.
                mi = 0
                first_class = True
                for w in range(W):
                    th = per_class[w]
                    if len(th) == 0:
                        continue
                    vector.tensor_scalar(pacc.ap(), va, float(th[0]), None,
                                         mybir.AluOpType.is_gt)
                    for t_j in th[1:]:
                        vector.scalar_tensor_tensor(
                            pacc.ap(), va, float(t_j), pacc.ap(),
                            mybir.AluOpType.is_gt, mybir.AluOpType.add)
                    wv = float(np.float32(vals[w] + DELTA))
                    if first_class:
                        vector.tensor_scalar(mrg[mi].ap(), pacc.ap(), wv, None,
                                             mybir.AluOpType.mult)
                        first_class = False
                    else:
                        vector.scalar_tensor_tensor(
                            mrg[1 - mi].ap(), pacc.ap(), wv, mrg[mi].ap(),
                            mybir.AluOpType.mult, mybir.AluOpType.add)
                        mi = 1 - mi
                # next tile's v while GPSIMD may still be running this tile
                if t + 1 < NTILES:
                    vector.wait_ge(dma_in_sem, 32 * (t + 2))
                    vector.tensor_tensor(v_sb[(t + 1) % 2].ap(),
                                         a_sb[(t + 1) % 2].ap(),
                                         b_sb[(t + 1) % 2].ap(),
                                         mybir.AluOpType.subtract).then_inc(v_sem, 1)
                # join GPSIMD's weighted partial sum
                if gps_list:
                    vector.wait_ge(gp_sem, t + 1)
                    vector.tensor_tensor(tmp_a.ap(), mrg[mi].ap(), mg_f[t % 2].ap(),
                                         mybir.AluOpType.add)
                    merged_ap = tmp_a.ap()
                else:
                    merged_ap = mrg[mi].ap()
                # extraction
                vector.tensor_scalar(tmp_b.ap(), merged_ap, 1.0 / Q, None,
                                     mybir.AluOpType.mult)
                vector.tensor_copy(si_sb[t % 2].ap(), tmp_b.ap())
                vector.tensor_copy(tmp_a.ap(), si_sb[t % 2].ap())
                vector.tensor_tensor(v_sb[t % 2].ap(), tmp_b.ap(), tmp_a.ap(),
                                     mybir.AluOpType.subtract)
                vector.tensor_scalar(si_sb[t % 2].ap(), v_sb[t % 2].ap(),
                                     Q / DELTA, None, mybir.AluOpType.mult)
                vector.tensor_scalar(tmp_b.ap(), tmp_a.ap(), Q, uv0_f,
                                     mybir.AluOpType.mult, mybir.AluOpType.add)
                vector.tensor_tensor(d_sb[t % 2].ap(), tmp_b.ap(), b_sb[t % 2].ap(),
                                     mybir.AluOpType.add).then_inc(cmp_sem, 1)

    return nc


# --------------------------------------------------------------------------
# Public entry point
# --------------------------------------------------------------------------
_CACHE: dict[bytes, bass.Bass] = {}


def _get_nc(uv: np.ndarray) -> bass.Bass:
    key = uv.tobytes()
    if key not in _CACHE:
        c, cls, vals, W = _plan(uv)
        _CACHE[key] = _build(c, cls, vals, W, float(np.float32(uv[0])))
    return _CACHE[key]


def kernel(inputs: np.ndarray, means: np.ndarray, unique_values: np.ndarray):
    inputs = np.ascontiguousarray(np.asarray(inputs, dtype=np.float32))
    means = np.ascontiguousarray(np.asarray(means, dtype=np.float32))
    uv = np.ascontiguousarray(np.asarray(unique_values, dtype=np.float32))

    nc = _get_nc(uv)

    bpc = B // NCORES
    in_maps = []
    for cid in range(NCORES):
        a = inputs[cid * bpc:(cid + 1) * bpc].reshape(P, FREE_PER_PART)
        b = means[cid * bpc:(cid + 1) * bpc].reshape(P, FREE_PER_PART)
        in_maps.append({"a": np.ascontiguousarray(a),
                        "b": np.ascontiguousarray(b)})

    res = run_bass_kernel_spmd(nc, in_maps, core_ids=list(range(NCORES)))

    dq = np.empty((B, CC, HH, WW), dtype=np.float32)
    sym = np.empty((B, CC, HH, WW), dtype=np.int32)
    for cid in range(NCORES):
        r = res.results[cid]
        dq[cid * bpc:(cid + 1) * bpc] = r["dq"].reshape(bpc, CC, HH, WW)
        sym[cid * bpc:(cid + 1) * bpc] = r["sym"].reshape(bpc, CC, HH, WW)
    return dq, sym
